# revision 1
# baseline (speedup 1.0000x reference)
"""DeepseekV2 MLA attention on 8 Trainium2 NeuronCores.

Sharding: token-split A-projections -> AllGather(latents) -> head-split
(4 heads/core) B-projections + causal attention -> AllGather(attn out) ->
D-column-split output projection. Layouts are d-major (feature dim on the
SBUF partition axis) so no on-device transposes are needed; the host
pre-transposes h and re-orders weight columns instead.

Precision: bf16 matmul inputs with fp32 PSUM accumulation throughout;
rmsnorm statistics, softmax (exp, denominators, rescale) and all staging
run in fp32/fp32r. Measured end-to-end relative error vs the fp32 jax
reference: ~3.9e-3.
"""
import math

import numpy as np
import ml_dtypes

import concourse.bass as bass
import concourse.mybir as mybir
from concourse.tile import TileContext
from concourse import bass_utils

# ---------------------------------------------------------------------------
# Walrus workaround: this container's walrus accepts at most ONE sync-wait
# per TPB instruction, but Tile attaches several (tail Drain, LDWEIGHTS...).
# Split: keep the last wait, move the rest onto preceding same-engine NOPs.
# ---------------------------------------------------------------------------
import concourse.tile as _tile_mod

_orig_sched = _tile_mod.TileContext.schedule_and_allocate
_nopctr = [0]


def _split_multiwait(nc):
    for fn in nc.m.functions:
        for blk in fn.blocks:
            insts = blk.instructions
            if not any(
                i.sync_info and i.sync_info.on_wait and len(i.sync_info.on_wait) > 1
                for i in insts
            ):
                continue
            out = []
            for ins in insts:
                si = ins.sync_info
                if si and si.on_wait and len(si.on_wait) > 1:
                    waits = list(si.on_wait)
                    for w in waits[:-1]:
                        _nopctr[0] += 1
                        nop = mybir.InstNoOp(name=f"I-mws-{_nopctr[0]}", ins=[], outs=[])
                        nop.engine = ins.engine
                        nop.sync_info = mybir.SyncInfo(on_wait=[w], on_update=[])
                        out.append(nop)
                    ins.sync_info = mybir.SyncInfo(
                        on_wait=[waits[-1]], on_update=list(si.on_update or [])
                    )
                out.append(ins)
            blk.instructions = out


def _patched_sched(self, *a, **k):
    res = _orig_sched(self, *a, **k)
    _split_multiwait(self.nc)
    return res


if getattr(_tile_mod.TileContext.schedule_and_allocate, "__name__", "") != "_patched_sched":
    _tile_mod.TileContext.schedule_and_allocate = _patched_sched


# ---------------------------------------------------------------------------
T, D, H = 2048, 5120, 32
NOPE, ROPE, QK = 128, 64, 192
KVR, QR, VH = 512, 1536, 128
EPS, THETA = 1e-6, 10000.0
NCORES = 8
HL = H // NCORES          # 4 heads per core
TC = T // NCORES          # 256 tokens per core
LAT = KVR + ROPE          # 576
AGR = QR + LAT            # 2112 rows in allgather-1
DCOL = D // NCORES        # 640 output columns per core

F32 = mybir.dt.float32
F32R = mybir.dt.float32r
BF16 = mybir.dt.bfloat16
AF = mybir.ActivationFunctionType
MUL = mybir.AluOpType.mult
ADD = mybir.AluOpType.add
SUB = mybir.AluOpType.subtract

TRACE = [False]          # test.py sets TRACE[0]=True to profile
LAST_RESULT = [None]     # BassKernelResults stashed here for test.py

_cache = {}


def _phase_a(nc, tc, io, consts_t, ag1a_in, ag1b_in):
    """Token-split A projections (bf16), rmsnorms, k_pe rope -> ag1_in."""
    ones_c, ones_r = consts_t["ones_c"], consts_t["ones_r"]
    cosa_sb, sina_sb, bias_sb = (consts_t["cosa_sb"], consts_t["sina_sb"],
                                 consts_t["bias_sb"])
    with (
        tc.tile_pool(name="a_ht", bufs=1) as a_ht,
        tc.tile_pool(name="a_w", bufs=3) as a_w,
        tc.tile_pool(name="a_st", bufs=1) as a_st,
        tc.tile_pool(name="a_tmp", bufs=3) as a_tmp,
        tc.tile_pool(name="a_ps", bufs=2, space="PSUM") as a_ps,
        tc.tile_pool(name="a_ss", bufs=1, space="PSUM") as a_ss,
    ):
        ht_sb = a_ht.tile([128, 40 * TC], BF16, name="ht_sb")
        htv = ht_sb[:].rearrange("p (k t) -> p k t", k=40)
        nc.sync.dma_start(htv, io["hT"][:].rearrange("(k p) t -> p k t", p=128))
        stage = a_st.tile([128, 17 * TC], F32R, name="stage")
        ss_q = a_ss.tile([1, TC], F32, name="ss_q")
        ss_kv = a_ss.tile([1, TC], F32, name="ss_kv")

        for m in range(17):
            mrows = 64 if m == 16 else 128
            # one column-chunk DMA: all 40 k-chunks of this m column
            wt = a_w.tile([128, 40 * 128], BF16, name=f"a_w_{m}", tag="aw")
            wtv = wt[:].rearrange("p (k c) -> p k c", k=40)
            if m < 12:
                nc.sync.dma_start(
                    wtv[:, :, :mrows],
                    io["wqa"][:].rearrange("(k p) q -> p k q", p=128)[
                        :, :, m * 128:(m + 1) * 128])
            else:
                nc.sync.dma_start(
                    wtv[:, :, :mrows],
                    io["wkva"][:].rearrange("(k p) q -> p k q", p=128)[
                        :, :, (m - 12) * 128:(m - 12) * 128 + mrows])
            ps = a_ps.tile([128, TC], F32, name=f"a_ps_{m}", tag="aps")
            for k in range(40):
                nc.tensor.matmul(ps[:mrows, :], wtv[:, k, :mrows], htv[:, k, :],
                                 start=(k == 0), stop=(k == 39))
            st = stage[:, m * TC:(m + 1) * TC]
            if m < 12:
                nc.vector.tensor_copy(st, ps[:])
                sq = a_tmp.tile([128, TC], F32R, name=f"sq_{m}", tag="sq")
                nc.scalar.activation(sq[:], st, AF.Square)
                nc.tensor.matmul(ss_q[:], ones_c, sq[:],
                                 start=(m == 0), stop=(m == 11))
            elif m < 16:
                nc.vector.tensor_scalar(st, ps[:], bias_sb[:, m - 12:m - 11],
                                        None, op0=ADD)
                sq = a_tmp.tile([128, TC], F32R, name=f"sq_{m}", tag="sq")
                nc.scalar.activation(sq[:], st, AF.Square)
                nc.tensor.matmul(ss_kv[:], ones_c, sq[:],
                                 start=(m == 12), stop=(m == 15))
            else:
                nc.vector.tensor_scalar(st[:64, :], ps[:64, :],
                                        bias_sb[:64, 4:5], None, op0=ADD)

        # rms scales: 1/sqrt(mean(ss) + eps) broadcast to 128 partitions
        bcs = {}
        for key, ss, nfeat in (("q", ss_q, QR), ("kv", ss_kv, KVR)):
            ms = a_tmp.tile([1, TC], F32R, name=f"ms_{key}", tag="ms")
            nc.vector.tensor_scalar(ms[:], ss[:], 1.0 / nfeat, EPS,
                                    op0=MUL, op1=ADD)
            sq2 = a_tmp.tile([1, TC], F32R, name=f"sqr_{key}", tag="sqr")
            nc.scalar.activation(sq2[:], ms[:], AF.Sqrt)
            rs = a_tmp.tile([1, TC], F32R, name=f"rs_{key}", tag="rs")
            with nc.allow_low_precision(reason="f32r holds full fp32 bits"):
                nc.vector.reciprocal(rs[:], sq2[:])
            bps = a_ps.tile([128, TC], F32, name=f"bps_{key}", tag="bps")
            nc.tensor.matmul(bps[:], ones_r[:1, :], rs[:], start=True, stop=True)
            bc = a_tmp.tile([128, TC], F32R, name=f"bc_{key}", tag=f"bc{key}")
            nc.vector.tensor_copy(bc[:], bps[:])
            bcs[key] = bc

        for m in range(16):
            st = stage[:, m * TC:(m + 1) * TC]
            sc = a_tmp.tile([128, TC], BF16, name=f"sc_{m}", tag="sc")
            nc.vector.tensor_tensor(sc[:], st, bcs["q" if m < 12 else "kv"][:],
                                    op=MUL)
            if m < 8:
                nc.sync.dma_start(ag1a_in[m * 128:(m + 1) * 128, :], sc[:])
            else:
                nc.sync.dma_start(ag1b_in[(m - 8) * 128:(m - 7) * 128, :], sc[:])

        # k_pe rope (no norm) -> rows 2048:2112
        st = stage[:, 16 * TC:17 * TC]
        rp = a_tmp.tile([64, TC], BF16, name="rp_kpe")
        t1 = a_tmp.tile([32, TC], F32R, name="rt1", tag="rt1")
        t2 = a_tmp.tile([32, TC], F32R, name="rt2", tag="rt2")
        x1, x2 = st[0:32, :], st[32:64, :]
        nc.vector.tensor_tensor(t1[:], x1, cosa_sb[0:32, :], op=MUL)
        nc.vector.tensor_tensor(t2[:], x2, sina_sb[32:64, :], op=MUL)
        nc.vector.tensor_tensor(rp[0:32, :], t1[:], t2[:], op=SUB)
        nc.vector.tensor_tensor(t1[:], x1, sina_sb[0:32, :], op=MUL)
        nc.vector.tensor_tensor(t2[:], x2, cosa_sb[32:64, :], op=MUL)
        nc.vector.tensor_tensor(rp[32:64, :], t1[:], t2[:], op=ADD)
        nc.sync.dma_start(ag1b_in[1024:1088, :], rp[:])


def _phase_b(nc, tc, io, ag1bv, ktv, vv, kpe_sb):
    """Head-split k_nope^T and v projections from the gathered latents."""
    with (
        tc.tile_pool(name="b_kva", bufs=1) as b_kva,
        tc.tile_pool(name="b_w", bufs=4) as b_w,
        tc.tile_pool(name="b_ps", bufs=2, space="PSUM") as b_ps,
    ):
        kva_sb = b_kva.tile([128, 4 * T], BF16, name="kva_sb")
        kvav = kva_sb[:].rearrange("p (k t) -> p k t", k=4)
        for k in range(4):
            nc.sync.dma_start(
                kvav[:, k, :].rearrange("p (r t) -> p r t", r=NCORES),
                ag1bv[512 + k * 128:512 + (k + 1) * 128])
        nc.sync.dma_start(
            kpe_sb[:].rearrange("p (r t) -> p r t", r=NCORES),
            ag1bv[1024:1088])

        wk_sb = b_w.tile([128, 4 * 512], BF16, name="wk_sb", tag="wkw")
        wkv_ = wk_sb[:].rearrange("p (k c) -> p k c", k=4)
        nc.sync.dma_start(wkv_, io["wkvbk"][:].rearrange("(k p) c -> p k c", p=128))
        for j in range(HL):
            for qb in range(4):
                ps = b_ps.tile([128, 512], F32, name=f"psk_{j}_{qb}", tag="psk")
                for k in range(4):
                    nc.tensor.matmul(ps[:], wkv_[:, k, j * 128:(j + 1) * 128],
                                     kvav[:, k, qb * 512:(qb + 1) * 512],
                                     start=(k == 0), stop=(k == 3))
                nc.vector.tensor_copy(ktv[:, j, qb * 512:(qb + 1) * 512], ps[:])

        wv_sb = b_w.tile([128, 4 * 512], BF16, name="wv_sb", tag="wvw")
        wvv = wv_sb[:].rearrange("p (k c) -> p k c", k=4)
        nc.sync.dma_start(wvv, io["wkvbv"][:].rearrange("(k p) c -> p k c", p=128))
        for mt in range(16):
            ps = b_ps.tile([128, 512], F32, name=f"psv_{mt}", tag="psv")
            for k in range(4):
                nc.tensor.matmul(ps[:], kvav[:, k, mt * 128:(mt + 1) * 128],
                                 wvv[:, k, :], start=(k == 0), stop=(k == 3))
            nc.vector.tensor_copy(vv[:, mt, :], ps[:])


def _phase_q(nc, tc, io, ag1av, ag1bv, qt_spill):
    """Head-split q^T projection with rope, into the resident qT tile."""
    with (
        tc.tile_pool(name="c_qa", bufs=1) as c_qa,
        tc.tile_pool(name="c_tab", bufs=1) as c_tab,
        tc.tile_pool(name="c_w", bufs=3) as c_w,
        tc.tile_pool(name="c_tmp", bufs=3) as c_tmp,
        tc.tile_pool(name="c_ps", bufs=2, space="PSUM") as c_ps,
    ):
        cos_sb = c_tab.tile([128, T], F32R, name="cos_sb")
        sin_sb = c_tab.tile([128, T], F32R, name="sin_sb")
        nc.sync.dma_start(cos_sb[:], io["cosT"][:])
        nc.sync.dma_start(sin_sb[:], io["sinT"][:])
        qa_sb = c_qa.tile([128, 12 * T], BF16, name="qa_sb")
        qav = qa_sb[:].rearrange("p (k t) -> p k t", k=12)
        for k in range(12):
            srcv = (ag1av[k * 128:(k + 1) * 128] if k < 8 else
                    ag1bv[(k - 8) * 128:(k - 7) * 128])
            nc.sync.dma_start(
                qav[:, k, :].rearrange("p (r t) -> p r t", r=NCORES), srcv)
        for m in range(6):
            wt = c_w.tile([128, 12 * 128], BF16, name=f"cw_{m}", tag="cw")
            wtv = wt[:].rearrange("p (k c) -> p k c", k=12)
            nc.sync.dma_start(
                wtv, io["wqb"][:].rearrange("(k p) c -> p k c", p=128)[
                    :, :, m * 128:(m + 1) * 128])
            pss = [c_ps.tile([128, 512], F32, name=f"psq_{m}_{qb}", tag=f"psq{qb}")
                   for qb in range(4)]
            for k in range(12):
                for qb in range(4):
                    nc.tensor.matmul(pss[qb][:], wtv[:, k, :],
                                     qav[:, k, qb * 512:(qb + 1) * 512],
                                     start=(k == 0), stop=(k == 11))
            for qb in range(4):
                st = c_tmp.tile([128, 512], BF16, name=f"cst_{m}_{qb}", tag="cst")
                if m < 4:
                    nc.vector.tensor_copy(st[:], pss[qb][:])
                else:
                    cs = cos_sb[:, qb * 512:(qb + 1) * 512]
                    sn = sin_sb[:, qb * 512:(qb + 1) * 512]
                    for half in range(2):
                        r0 = 64 * half
                        x1 = pss[qb][r0:r0 + 32, :]
                        x2 = pss[qb][r0 + 32:r0 + 64, :]
                        t1 = c_tmp.tile([32, 512], F32R,
                                        name=f"ct1_{m}_{qb}_{half}", tag="ct1")
                        t2 = c_tmp.tile([32, 512], F32R,
                                        name=f"ct2_{m}_{qb}_{half}", tag="ct2")
                        nc.vector.tensor_tensor(t1[:], x1, cs[r0:r0 + 32, :],
                                                op=MUL)
                        nc.vector.tensor_tensor(t2[:], x2,
                                                sn[r0 + 32:r0 + 64, :], op=MUL)
                        nc.vector.tensor_tensor(st[r0:r0 + 32, :],
                                                t1[:], t2[:], op=SUB)
                        nc.vector.tensor_tensor(t1[:], x1, sn[r0:r0 + 32, :],
                                                op=MUL)
                        nc.vector.tensor_tensor(t2[:], x2,
                                                cs[r0 + 32:r0 + 64, :], op=MUL)
                        nc.vector.tensor_tensor(st[r0 + 32:r0 + 64, :],
                                                t1[:], t2[:], op=ADD)
                nc.sync.dma_start(
                    qt_spill[m * 128:(m + 1) * 128, qb * 512:(qb + 1) * 512],
                    st[:])


def _phase_attn(nc, tc, qt_spill, ag2_ins, ag2_outs, ktv, vv, kpe_sb, consts_t):
    """Causal attention, two heads interleaved per pass; bf16 out -> ag2_in."""
    ones_c, ones_r, tri_sb = (consts_t["ones_cb"], consts_t["ones_r"],
                              consts_t["tri_b"])
    with (
        tc.tile_pool(name="t_qf", bufs=3) as t_qf,
        tc.tile_pool(name="t_p", bufs=8) as t_p,
        tc.tile_pool(name="t_o", bufs=2) as t_o,
        tc.tile_pool(name="t_ps", bufs=3, space="PSUM") as t_ps,
        tc.tile_pool(name="t_bc", bufs=1, space="PSUM") as t_bc,
        tc.tile_pool(name="t_acc", bufs=1, space="PSUM") as t_acc,
    ):
        for qb in range(4):
            for jp in range(HL // 2):
                js = (2 * jp, 2 * jp + 1)
                qf = {}
                dens, ots = {}, {}
                for s, j in enumerate(js):
                    qfn = t_qf.tile([128, 512], BF16, name=f"qfn_{qb}_{j}",
                                    tag=f"qfn{s}")
                    qfp = t_qf.tile([64, 512], BF16, name=f"qfp_{qb}_{j}",
                                    tag=f"qfp{s}")
                    nc.sync.dma_start(
                        qfn[:], qt_spill[j * 128:(j + 1) * 128,
                                         qb * 512:(qb + 1) * 512])
                    pchunk, phalf = 4 + j // 2, j % 2
                    rr = pchunk * 128 + 64 * phalf
                    nc.sync.dma_start(
                        qfp[:], qt_spill[rr:rr + 64, qb * 512:(qb + 1) * 512])
                    qf[j] = (qfn, qfp)
                    dens[j] = t_acc.tile([1, 512], F32, name=f"den_{qb}_{j}",
                                         tag=f"den{s}")
                    ots[j] = t_acc.tile([128, 512], F32, name=f"ot_{qb}_{j}",
                                        tag=f"ot{s}")
                kmax = 4 * qb + 4
                for kk in range(kmax):
                    o = kk - 4 * qb
                    c0 = max(0, o) * 128
                    pts = {}
                    for s, j in enumerate(js):
                        qfn, qfp = qf[j]
                        sT = t_ps.tile([128, 512], F32,
                                       name=f"sT_{qb}_{j}_{kk}", tag="sT")
                        nc.tensor.matmul(sT[:, c0:512],
                                         ktv[:, j, kk * 128:(kk + 1) * 128],
                                         qfn[:, c0:512], start=True, stop=False)
                        nc.tensor.matmul(sT[:, c0:512],
                                         kpe_sb[:, kk * 128:(kk + 1) * 128],
                                         qfp[:, c0:512], start=False, stop=True)
                        pT = t_p.tile([128, 512], BF16,
                                      name=f"pT_{qb}_{j}_{kk}", tag="pT")
                        nc.scalar.activation(pT[:, c0:512], sT[:, c0:512],
                                             AF.Exp)
                        if o >= 0:
                            nc.vector.tensor_tensor(pT[:, c0:c0 + 128],
                                                    pT[:, c0:c0 + 128],
                                                    tri_sb[:], op=MUL)
                        pts[j] = pT
                    for j in js:
                        pT = pts[j]
                        nc.tensor.matmul(dens[j][:, c0:512], ones_c,
                                         pT[:, c0:512],
                                         start=(kk == 0), stop=(kk == kmax - 1))
                        nc.tensor.matmul(ots[j][:, c0:512],
                                         vv[:, kk, j * 128:(j + 1) * 128],
                                         pT[:, c0:512],
                                         start=(kk == 0), stop=(kk == kmax - 1))
                for s, j in enumerate(js):
                    den, ot = dens[j], ots[j]
                    rden = t_o.tile([1, 512], F32R, name=f"rden_{qb}_{j}",
                                    tag=f"rden{s}")
                    with nc.allow_low_precision(reason="f32r = fp32 bits"):
                        nc.vector.reciprocal(rden[:], den[:])
                    bcp = t_bc.tile([128, 512], F32, name=f"bcp_{qb}_{j}",
                                    tag="bcp")
                    nc.tensor.matmul(bcp[:], ones_r[:1, :], rden[:],
                                     start=True, stop=True)
                    bcs = t_o.tile([128, 512], F32R, name=f"bcs_{qb}_{j}",
                                   tag=f"bcs{s}")
                    nc.vector.tensor_copy(bcs[:], bcp[:])
                    obf = t_o.tile([128, 512], BF16, name=f"obf_{qb}_{j}",
                                   tag=f"obf{s}")
                    nc.vector.tensor_tensor(obf[:], ots[j][:], bcs[:], op=MUL)
                    nc.sync.dma_start(
                        ag2_ins[qb][j * 128:(j + 1) * 128, :], obf[:])
            nc.gpsimd.collective_compute(
                "AllGather", mybir.AluOpType.bypass,
                ins=[ag2_ins[qb][:]], outs=[ag2_outs[qb][:]],
                replica_groups=[list(range(NCORES))],
            )


def _phase_out(nc, tc, io, ag2_outs, wov):
    """D-column-split output projection (bf16); wo preloaded upstream."""
    with (
        tc.tile_pool(name="o_a", bufs=2) as o_a,
        tc.tile_pool(name="o_st", bufs=3) as o_st,
        tc.tile_pool(name="o_ps", bufs=3, space="PSUM") as o_ps,
    ):
        for tq in range(4):
            oa = o_a.tile([128, 32 * 512], BF16, name=f"oa_{tq}", tag="oa")
            oav = oa[:].rearrange("p (k t) -> p k t", k=32)
            nc.sync.dma_start(
                oav, ag2_outs[tq][:].rearrange("(k p) t -> p k t", p=128))
            for d in range(5):
                ps = o_ps.tile([128, 512], F32, name=f"ops_{tq}_{d}", tag="ops")
                for k in range(32):
                    nc.tensor.matmul(ps[:], wov[:, k, d * 128:(d + 1) * 128],
                                     oav[:, k, :], start=(k == 0), stop=(k == 31))
                st = o_st.tile([128, 512], F32, name=f"ost_{tq}_{d}", tag="ost")
                nc.vector.tensor_copy(st[:], ps[:])
                nc.sync.dma_start(
                    io["outT"][d * 128:(d + 1) * 128,
                               tq * 512:(tq + 1) * 512], st[:])


def _build():
    nc = bass.Bass("TRN2", target_bir_lowering=False, debug=False,
                   num_devices=NCORES)
    io = {
        "hT": nc.dram_tensor("hT", [D, TC], BF16, kind="ExternalInput"),
        "wqa": nc.dram_tensor("wqa", [D, QR], BF16, kind="ExternalInput"),
        "wkva": nc.dram_tensor("wkva", [D, LAT], BF16, kind="ExternalInput"),
        "biask": nc.dram_tensor("biask", [128, 5], F32, kind="ExternalInput"),
        "wqb": nc.dram_tensor("wqb", [QR, 6 * 128], BF16, kind="ExternalInput"),
        "wkvbk": nc.dram_tensor("wkvbk", [KVR, HL * NOPE], BF16,
                                kind="ExternalInput"),
        "wkvbv": nc.dram_tensor("wkvbv", [KVR, HL * VH], BF16,
                                kind="ExternalInput"),
        "wo": nc.dram_tensor("wo", [H * VH, DCOL], BF16, kind="ExternalInput"),
        "cosT": nc.dram_tensor("cosT", [128, T], F32R, kind="ExternalInput"),
        "sinT": nc.dram_tensor("sinT", [128, T], F32R, kind="ExternalInput"),
        "cosA": nc.dram_tensor("cosA", [128, TC], F32R, kind="ExternalInput"),
        "sinA": nc.dram_tensor("sinA", [128, TC], F32R, kind="ExternalInput"),
        "tri": nc.dram_tensor("tri", [128, 128], F32R, kind="ExternalInput"),
        "onesin": nc.dram_tensor("onesin", [128, 128], F32R, kind="ExternalInput"),
        "outT": nc.dram_tensor("outT", [DCOL, T], F32, kind="ExternalOutput"),
    }

    with TileContext(nc) as tc:
        with (
            tc.tile_pool(name="dram", bufs=1, space="DRAM") as dram,
            tc.tile_pool(name="consts", bufs=1) as consts,
        ):
            ag1a_in = dram.tile([1024, TC], BF16, name="ag1a_in")
            ag1a_out = dram.tile([NCORES * 1024, TC], BF16, addr_space="Shared",
                                 name="ag1a_out")
            ag1b_in = dram.tile([1088, TC], BF16, name="ag1b_in")
            ag1b_out = dram.tile([NCORES * 1088, TC], BF16, addr_space="Shared",
                                 name="ag1b_out")
            ag2_ins = [dram.tile([HL * VH, 512], BF16, name=f"ag2_in_{qb}")
                       for qb in range(4)]
            ag2_outs = [dram.tile([H * VH, 512], BF16, addr_space="Shared",
                                  name=f"ag2_out_{qb}") for qb in range(4)]

            consts_t = {}
            ones_sb = consts.tile([128, 128], F32R, name="ones_sb")
            nc.sync.dma_start(ones_sb[:], io["onesin"][:])
            consts_t["ones_c"] = ones_sb[:, 0:1]
            consts_t["ones_r"] = ones_sb
            ones_b = consts.tile([128, 1], BF16, name="ones_b")
            nc.vector.tensor_copy(ones_b[:], ones_sb[:, 0:1])
            consts_t["ones_cb"] = ones_b[:]
            trib = consts.tile([128, 128], BF16, name="trib")
            consts_t["tri_b"] = trib
            for nm, srcn, shp in (("tri_sb", "tri", [128, 128]),
                                  ("cosa_sb", "cosA", [128, TC]),
                                  ("sina_sb", "sinA", [128, TC]),
                                  ):
                consts_t[nm] = consts.tile(shp, F32R, name=nm)
                nc.sync.dma_start(consts_t[nm][:], io[srcn][:])
            consts_t["bias_sb"] = consts.tile([128, 5], F32, name="bias_sb")
            nc.sync.dma_start(consts_t["bias_sb"][:], io["biask"][:])
            nc.vector.tensor_copy(trib[:], consts_t["tri_sb"][:])

            with nc.named_scope("phase_a"):
                _phase_a(nc, tc, io, consts_t, ag1a_in, ag1b_in)

            with nc.named_scope("ag1"):
                nc.gpsimd.collective_compute(
                    "AllGather", mybir.AluOpType.bypass,
                    ins=[ag1a_in[:]], outs=[ag1a_out[:]],
                    replica_groups=[list(range(NCORES))],
                )
                nc.gpsimd.collective_compute(
                    "AllGather", mybir.AluOpType.bypass,
                    ins=[ag1b_in[:]], outs=[ag1b_out[:]],
                    replica_groups=[list(range(NCORES))],
                )
            ag1av = ag1a_out[:].rearrange("(r a) t -> a r t", a=1024)
            ag1bv = ag1b_out[:].rearrange("(r a) t -> a r t", a=1088)

            qt_spill = dram.tile([6 * 128, T], BF16, name="qt_spill")
            with nc.named_scope("phase_q"):
                _phase_q(nc, tc, io, ag1av, ag1bv, qt_spill)
            with tc.tile_pool(name="persist", bufs=1) as persist:
                kt_sb = persist.tile([128, HL * T], BF16, name="kt_sb")
                ktv = kt_sb[:].rearrange("p (j t) -> p j t", j=HL)
                v_sb = persist.tile([128, 16 * 512], BF16, name="v_sb")
                vv = v_sb[:].rearrange("p (mt c) -> p mt c", mt=16)
                kpe_sb = persist.tile([64, T], BF16, name="kpe_sb")
                with nc.named_scope("phase_b"):
                    _phase_b(nc, tc, io, ag1bv, ktv, vv, kpe_sb)
                wo_sb = persist.tile([128, 32 * DCOL], BF16, name="wo_sb")
                wov = wo_sb[:].rearrange("p (k c) -> p k c", k=32)
                nc.sync.dma_start(
                    wov, io["wo"][:].rearrange("(k p) c -> p k c", p=128))
                with nc.named_scope("phase_attn"):
                    _phase_attn(nc, tc, qt_spill, ag2_ins, ag2_outs,
                                ktv, vv, kpe_sb, consts_t)

                with nc.named_scope("phase_out"):
                    _phase_out(nc, tc, io, ag2_outs, wov)
    return nc


def _get_nc():
    if "nc" not in _cache:
        _cache["nc"] = _build()
    return _cache["nc"]


def _prep(inputs):
    h = np.asarray(inputs["h"], np.float32)
    pos = np.asarray(inputs["position_ids"], np.int32)
    Wq_a = np.asarray(inputs["Wq_a"], np.float32)
    gq = np.asarray(inputs["gq"], np.float32)
    Wq_b = np.asarray(inputs["Wq_b"], np.float32)
    Wkv_a = np.asarray(inputs["Wkv_a"], np.float32)
    bkv_a = np.asarray(inputs["bkv_a"], np.float32)
    gkv = np.asarray(inputs["gkv"], np.float32)
    Wkv_b = np.asarray(inputs["Wkv_b"], np.float32)
    Wo = np.asarray(inputs["Wo"], np.float32)

    dperm = np.concatenate([np.arange(0, ROPE, 2), np.arange(1, ROPE, 2)])
    scale = np.float32(1.0 / math.sqrt(QK))

    hT = np.ascontiguousarray(h.T)                      # [D, T]
    wkva = Wkv_a.copy()
    wkva[:, KVR:] = Wkv_a[:, KVR + dperm]
    bias = bkv_a.copy()
    bias[KVR:] = bkv_a[KVR + dperm]
    bm = np.zeros((5, 128), np.float32)
    bm.reshape(-1)[:LAT] = bias
    biask = np.ascontiguousarray(bm.T)                  # [128, 5]

    wqb_eff = (Wq_b * gq[:, None]) * scale              # [QR, H*QK]
    wkvb_eff = Wkv_b * gkv[:, None]                     # [KVR, H*(NOPE+VH)]

    inv = THETA ** (-np.arange(0, ROPE, 2, dtype=np.float32) / ROPE)
    fr = pos.astype(np.float32)[:, None] * inv[None, :]  # [T, 32]
    cosT = np.ascontiguousarray(np.tile(np.cos(fr).T, (4, 1)))  # [128, T]
    sinT = np.ascontiguousarray(np.tile(np.sin(fr).T, (4, 1)))
    tri = np.triu(np.ones((128, 128), np.float32))
    wqa_b = Wq_a.astype(ml_dtypes.bfloat16)
    wkva_b = wkva.astype(ml_dtypes.bfloat16)

    bf16 = ml_dtypes.bfloat16
    in_maps = []
    for c in range(NCORES):
        heads = list(range(HL * c, HL * (c + 1)))
        qcols = [np.arange(hh * QK, hh * QK + NOPE) for hh in heads]
        for pair in range(2):
            for hh in heads[2 * pair:2 * pair + 2]:
                qcols.append(hh * QK + NOPE + dperm)
        kcols = np.concatenate(
            [np.arange(hh * (NOPE + VH), hh * (NOPE + VH) + NOPE)
             for hh in heads])
        vcols = np.concatenate(
            [np.arange(hh * (NOPE + VH) + NOPE, (hh + 1) * (NOPE + VH))
             for hh in heads])
        in_maps.append({
            "hT": np.ascontiguousarray(hT[:, c * TC:(c + 1) * TC]).astype(bf16),
            "wqa": wqa_b,
            "wkva": wkva_b,
            "biask": biask,
            "wqb": np.ascontiguousarray(wqb_eff[:, np.concatenate(qcols)]).astype(bf16),
            "wkvbk": np.ascontiguousarray(wkvb_eff[:, kcols]).astype(bf16),
            "wkvbv": np.ascontiguousarray(wkvb_eff[:, vcols]).astype(bf16),
            "wo": np.ascontiguousarray(Wo[:, c * DCOL:(c + 1) * DCOL]).astype(bf16),
            "cosT": cosT,
            "sinT": sinT,
            "cosA": np.ascontiguousarray(cosT[:, c * TC:(c + 1) * TC]),
            "sinA": np.ascontiguousarray(sinT[:, c * TC:(c + 1) * TC]),
            "tri": tri,
            "onesin": np.ones((128, 128), np.float32),
        })
    return in_maps


def kernel(**inputs):
    nc = _get_nc()
    in_maps = _prep(inputs)
    res = bass_utils.run_bass_kernel_spmd(
        nc, in_maps, core_ids=list(range(NCORES)), trace=TRACE[0])
    LAST_RESULT[0] = res
    out = np.empty((T, D), np.float32)
    for c in range(NCORES):
        out[:, c * DCOL:(c + 1) * DCOL] = res.results[c]["outT"].T
    return out



# revision 5
# speedup vs baseline: 1.1118x; 1.1118x over previous
"""DeepseekV2 MLA attention on 8 Trainium2 NeuronCores.

Sharding: token-split A-projections -> AllGather(latents) -> head-split
(4 heads/core) B-projections + causal attention -> AllGather(attn out) ->
D-column-split output projection. Layouts are d-major (feature dim on the
SBUF partition axis) so no on-device transposes are needed; the host
pre-transposes h and re-orders weight columns instead.

v2 schedule: the kv-latent chunks are computed first so their (small)
AllGather overlaps the q-latent matmuls, and the q AllGather overlaps
phase_b; qT stays resident in SBUF (no DRAM spill); attention runs all 4
heads per key-block pass with col-tiled denominator matmuls and a single
batched reciprocal per query block; the output projection for query block
t is interleaved with attention on block t+2 so ag2 latency hides.

Precision: bf16 matmul inputs with fp32 PSUM accumulation throughout;
rmsnorm statistics, softmax and all staging run in fp32/fp32r.
"""
import math

import numpy as np
import ml_dtypes

import concourse.bass as bass
import concourse.mybir as mybir
from concourse.tile import TileContext
from concourse import bass_utils

# ---------------------------------------------------------------------------
# Walrus workaround: this container's walrus accepts at most ONE sync-wait
# per TPB instruction, but Tile attaches several (tail Drain, LDWEIGHTS...).
# Split: keep the last wait, move the rest onto preceding same-engine NOPs.
# ---------------------------------------------------------------------------
import concourse.tile as _tile_mod

_orig_sched = _tile_mod.TileContext.schedule_and_allocate
_nopctr = [0]


def _split_multiwait(nc):
    for fn in nc.m.functions:
        for blk in fn.blocks:
            insts = blk.instructions
            if not any(
                i.sync_info and i.sync_info.on_wait and len(i.sync_info.on_wait) > 1
                for i in insts
            ):
                continue
            out = []
            for ins in insts:
                si = ins.sync_info
                if si and si.on_wait and len(si.on_wait) > 1:
                    waits = list(si.on_wait)
                    for w in waits[:-1]:
                        _nopctr[0] += 1
                        nop = mybir.InstNoOp(name=f"I-mws-{_nopctr[0]}", ins=[], outs=[])
                        nop.engine = ins.engine
                        nop.sync_info = mybir.SyncInfo(on_wait=[w], on_update=[])
                        out.append(nop)
                    ins.sync_info = mybir.SyncInfo(
                        on_wait=[waits[-1]], on_update=list(si.on_update or [])
                    )
                out.append(ins)
            blk.instructions = out


def _patched_sched(self, *a, **k):
    res = _orig_sched(self, *a, **k)
    _split_multiwait(self.nc)
    return res


if getattr(_tile_mod.TileContext.schedule_and_allocate, "__name__", "") != "_patched_sched":
    _tile_mod.TileContext.schedule_and_allocate = _patched_sched


# ---------------------------------------------------------------------------
T, D, H = 2048, 5120, 32
NOPE, ROPE, QK = 128, 64, 192
KVR, QR, VH = 512, 1536, 128
EPS, THETA = 1e-6, 10000.0
NCORES = 8
HL = H // NCORES          # 4 heads per core
TC = T // NCORES          # 256 tokens per core
LAT = KVR + ROPE          # 576
DCOL = D // NCORES        # 640 output columns per core

F32 = mybir.dt.float32
F32R = mybir.dt.float32r
BF16 = mybir.dt.bfloat16
AF = mybir.ActivationFunctionType
MUL = mybir.AluOpType.mult
ADD = mybir.AluOpType.add
SUB = mybir.AluOpType.subtract

TRACE = [False]          # test.py sets TRACE[0]=True to profile
LAST_RESULT = [None]     # BassKernelResults stashed here for test.py

_cache = {}


def _phase_a_chunks(nc, tc, io, consts_t, ah, ms, ss, bias_first):
    """Matmul+stage the given m-chunks of the A projections (token-split)."""
    htv, stage, a_w, a_ps, a_tmp = (ah["htv"], ah["stage"], ah["a_w"],
                                    ah["a_ps"], ah["a_tmp"])
    ones_c, bias_sb = consts_t["ones_c"], consts_t["bias_sb"]
    for i, m in enumerate(ms):
        mrows = 64 if m == 16 else 128
        wt = a_w.tile([128, 40 * 128], BF16, name=f"a_w_{m}", tag="aw")
        wtv = wt[:].rearrange("p (k c) -> p k c", k=40)
        if m < 12:
            nc.sync.dma_start(
                wtv[:, :, :mrows],
                io["wqa"][:].rearrange("(k p) q -> p k q", p=128)[
                    :, :, m * 128:(m + 1) * 128])
        else:
            nc.sync.dma_start(
                wtv[:, :, :mrows],
                io["wkva"][:].rearrange("(k p) q -> p k q", p=128)[
                    :, :, (m - 12) * 128:(m - 12) * 128 + mrows])
        ps = a_ps.tile([128, TC], F32, name=f"a_ps_{m}", tag="aps")
        for k in range(40):
            nc.tensor.matmul(ps[:mrows, :], wtv[:, k, :mrows], htv[:, k, :],
                             start=(k == 0), stop=(k == 39))
        st = stage[:, m * TC:(m + 1) * TC]
        if m < 16:
            if m < 12:
                nc.vector.tensor_copy(st, ps[:])
            else:
                nc.vector.tensor_scalar(st, ps[:], bias_sb[:, m - 12:m - 11],
                                        None, op0=ADD)
            sq = a_tmp.tile([128, TC], F32R, name=f"sq_{m}", tag="sq")
            nc.scalar.activation(sq[:], st, AF.Square)
            nc.tensor.matmul(ss[:], ones_c, sq[:],
                             start=(i == 0), stop=(i == len(ms) - 1 - bias_first))
        else:
            nc.vector.tensor_scalar(st[:64, :], ps[:64, :],
                                    bias_sb[:64, 4:5], None, op0=ADD)


def _phase_a_scale(nc, tc, consts_t, ah, ss, nfeat, key):
    """rms scale 1/sqrt(mean(ss)+eps) broadcast to 128 partitions."""
    a_ps, a_tmp = ah["a_ps"], ah["a_tmp"]
    ones_r = consts_t["ones_r"]
    msc = a_tmp.tile([1, TC], F32R, name=f"ms_{key}", tag="ms")
    nc.vector.tensor_scalar(msc[:], ss[:], 1.0 / nfeat, EPS, op0=MUL, op1=ADD)
    sq2 = a_tmp.tile([1, TC], F32R, name=f"sqr_{key}", tag="sqr")
    nc.scalar.activation(sq2[:], msc[:], AF.Sqrt)
    rs = a_tmp.tile([1, TC], F32R, name=f"rs_{key}", tag="rs")
    with nc.allow_low_precision(reason="f32r holds full fp32 bits"):
        nc.vector.reciprocal(rs[:], sq2[:])
    bps = a_ps.tile([128, TC], F32, name=f"bps_{key}", tag="bps")
    nc.tensor.matmul(bps[:], ones_r[:1, :], rs[:], start=True, stop=True)
    bc = a_tmp.tile([128, TC], F32R, name=f"bc_{key}", tag=f"bc{key}")
    nc.vector.tensor_copy(bc[:], bps[:])
    return bc


def _phase_a_kv(nc, tc, io, consts_t, ah, ag1kv_in):
    """kv-latent chunks first: norm, kpe rope -> ag1kv_in (fired early)."""
    stage, a_tmp = ah["stage"], ah["a_tmp"]
    cosa_sb, sina_sb = consts_t["cosa_sb"], consts_t["sina_sb"]
    ss_kv = ah["ss_kv"]
    _phase_a_chunks(nc, tc, io, consts_t, ah, [12, 13, 14, 15, 16], ss_kv, 1)
    bc = _phase_a_scale(nc, tc, consts_t, ah, ss_kv, KVR, "kv")
    for m in range(12, 16):
        st = stage[:, m * TC:(m + 1) * TC]
        sc = a_tmp.tile([128, TC], BF16, name=f"sc_{m}", tag="sc")
        nc.vector.tensor_tensor(sc[:], st, bc[:], op=MUL)
        nc.sync.dma_start(ag1kv_in[(m - 12) * 128:(m - 11) * 128, :], sc[:])
    # k_pe rope (no norm) -> rows 512:576
    st = stage[:, 16 * TC:17 * TC]
    rp = a_tmp.tile([64, TC], BF16, name="rp_kpe")
    t1 = a_tmp.tile([32, TC], F32R, name="rt1", tag="rt1")
    t2 = a_tmp.tile([32, TC], F32R, name="rt2", tag="rt2")
    x1, x2 = st[0:32, :], st[32:64, :]
    nc.vector.tensor_tensor(t1[:], x1, cosa_sb[0:32, :], op=MUL)
    nc.vector.tensor_tensor(t2[:], x2, sina_sb[32:64, :], op=MUL)
    nc.vector.tensor_tensor(rp[0:32, :], t1[:], t2[:], op=SUB)
    nc.vector.tensor_tensor(t1[:], x1, sina_sb[0:32, :], op=MUL)
    nc.vector.tensor_tensor(t2[:], x2, cosa_sb[32:64, :], op=MUL)
    nc.vector.tensor_tensor(rp[32:64, :], t1[:], t2[:], op=ADD)
    nc.sync.dma_start(ag1kv_in[512:576, :], rp[:])


def _phase_a_q(nc, tc, io, consts_t, ah, ag1q_in):
    """q-latent chunks: norm -> ag1q_in."""
    stage, a_tmp = ah["stage"], ah["a_tmp"]
    ss_q = ah["ss_q"]
    _phase_a_chunks(nc, tc, io, consts_t, ah, list(range(12)), ss_q, 0)
    bc = _phase_a_scale(nc, tc, consts_t, ah, ss_q, QR, "q")
    for m in range(12):
        st = stage[:, m * TC:(m + 1) * TC]
        sc = a_tmp.tile([128, TC], BF16, name=f"sc_{m}", tag="sc")
        nc.vector.tensor_tensor(sc[:], st, bc[:], op=MUL)
        nc.sync.dma_start(ag1q_in[m * 128:(m + 1) * 128, :], sc[:])


def _phase_b(nc, tc, io, ag1kvv, ktv, vv, kpe2):
    """Head-split k_nope^T and v projections from the gathered kv latents."""
    with (
        tc.tile_pool(name="b_kva", bufs=1) as b_kva,
        tc.tile_pool(name="b_w", bufs=1) as b_w,
        tc.tile_pool(name="b_ps", bufs=1, space="PSUM") as b_ps,
        tc.tile_pool(name="b_psv", bufs=2, space="PSUM") as b_psv,
    ):
        kva_sb = b_kva.tile([128, 4 * T], BF16, name="kva_sb")
        kvav = kva_sb[:].rearrange("p (k t) -> p k t", k=4)
        for k in range(4):
            nc.sync.dma_start(
                kvav[:, k, :].rearrange("p (r t) -> p r t", r=NCORES),
                ag1kvv[k * 128:(k + 1) * 128])
        # kpe duplicated into both partition halves so q-rope slices at
        # base 0 and base 64 both have a matching-base stationary operand
        for half in range(2):
            nc.sync.dma_start(
                kpe2[64 * half:64 * half + 64, :].rearrange(
                    "p (r t) -> p r t", r=NCORES),
                ag1kvv[512:576])

        wk_sb = b_w.tile([128, 4 * 512], BF16, name="wk_sb")
        wkv_ = wk_sb[:].rearrange("p (k c) -> p k c", k=4)
        nc.sync.dma_start(wkv_, io["wkvbk"][:].rearrange("(k p) c -> p k c", p=128))
        for j in range(HL):
            pss = [b_ps.tile([128, 512], F32, name=f"psk_{j}_{qb}", tag=f"psk{qb}")
                   for qb in range(4)]
            for k in range(4):
                for qb in range(4):
                    nc.tensor.matmul(pss[qb][:], wkv_[:, k, j * 128:(j + 1) * 128],
                                     kvav[:, k, qb * 512:(qb + 1) * 512],
                                     start=(k == 0), stop=(k == 3))
            for qb in range(4):
                nc.vector.tensor_copy(ktv[:, j, qb * 512:(qb + 1) * 512],
                                      pss[qb][:])

        wv_sb = b_w.tile([128, 4 * 512], BF16, name="wv_sb")
        wvv = wv_sb[:].rearrange("p (k c) -> p k c", k=4)
        nc.sync.dma_start(wvv, io["wkvbv"][:].rearrange("(k p) c -> p k c", p=128))
        for mt in range(16):
            ps = b_psv.tile([128, 512], F32, name=f"psv_{mt}", tag="psv")
            for k in range(4):
                nc.tensor.matmul(ps[:], kvav[:, k, mt * 128:(mt + 1) * 128],
                                 wvv[:, k, :], start=(k == 0), stop=(k == 3))
            nc.vector.tensor_copy(vv[:, mt, :], ps[:])


def _phase_q(nc, tc, io, ag1qv, qtv):
    """Head-split q^T projection with rope, written into the resident qT."""
    with (
        tc.tile_pool(name="c_qa", bufs=1) as c_qa,
        tc.tile_pool(name="c_tab", bufs=1) as c_tab,
        tc.tile_pool(name="c_w", bufs=3) as c_w,
        tc.tile_pool(name="c_tmp", bufs=3) as c_tmp,
        tc.tile_pool(name="c_ps", bufs=2, space="PSUM") as c_ps,
    ):
        cos_sb = c_tab.tile([128, T], F32R, name="cos_sb")
        sin_sb = c_tab.tile([128, T], F32R, name="sin_sb")
        nc.sync.dma_start(cos_sb[:], io["cosT"][:])
        nc.sync.dma_start(sin_sb[:], io["sinT"][:])
        qa_sb = c_qa.tile([128, 12 * T], BF16, name="qa_sb")
        qav = qa_sb[:].rearrange("p (k t) -> p k t", k=12)
        for k in range(12):
            nc.sync.dma_start(
                qav[:, k, :].rearrange("p (r t) -> p r t", r=NCORES),
                ag1qv[k * 128:(k + 1) * 128])
        for m in range(6):
            wt = c_w.tile([128, 12 * 128], BF16, name=f"cw_{m}", tag="cw")
            wtv = wt[:].rearrange("p (k c) -> p k c", k=12)
            nc.sync.dma_start(
                wtv, io["wqb"][:].rearrange("(k p) c -> p k c", p=128)[
                    :, :, m * 128:(m + 1) * 128])
            pss = [c_ps.tile([128, 512], F32, name=f"psq_{m}_{qb}", tag=f"psq{qb}")
                   for qb in range(4)]
            for k in range(12):
                for qb in range(4):
                    nc.tensor.matmul(pss[qb][:], wtv[:, k, :],
                                     qav[:, k, qb * 512:(qb + 1) * 512],
                                     start=(k == 0), stop=(k == 11))
            for qb in range(4):
                dst = qtv[:, m, qb * 512:(qb + 1) * 512]
                if m < 4:
                    nc.vector.tensor_copy(dst, pss[qb][:])
                else:
                    cs = cos_sb[:, qb * 512:(qb + 1) * 512]
                    sn = sin_sb[:, qb * 512:(qb + 1) * 512]
                    for half in range(2):
                        r0 = 64 * half
                        x1 = pss[qb][r0:r0 + 32, :]
                        x2 = pss[qb][r0 + 32:r0 + 64, :]
                        t1 = c_tmp.tile([32, 512], F32R,
                                        name=f"ct1_{m}_{qb}_{half}", tag="ct1")
                        t2 = c_tmp.tile([32, 512], F32R,
                                        name=f"ct2_{m}_{qb}_{half}", tag="ct2")
                        nc.vector.tensor_tensor(t1[:], x1, cs[r0:r0 + 32, :],
                                                op=MUL)
                        nc.vector.tensor_tensor(t2[:], x2,
                                                sn[r0 + 32:r0 + 64, :], op=MUL)
                        nc.vector.tensor_tensor(dst[r0:r0 + 32, :],
                                                t1[:], t2[:], op=SUB)
                        nc.vector.tensor_tensor(t1[:], x1, sn[r0:r0 + 32, :],
                                                op=MUL)
                        nc.vector.tensor_tensor(t2[:], x2,
                                                cs[r0 + 32:r0 + 64, :], op=MUL)
                        nc.vector.tensor_tensor(dst[r0 + 32:r0 + 64, :],
                                                t1[:], t2[:], op=ADD)


def _attn_block(nc, tc, qb, qtv, ktv, vv, kpe2, ag2_in, ag2_out, consts_t, ap):
    """Causal attention for one 512-query block, all 4 heads per key pass."""
    ones_r, ones_b, tri_b = (consts_t["ones_r"], consts_t["ones_cb"],
                             consts_t["tri_b"])
    t_p, t_o, t_ps, t_acc, t_den = (ap["t_p"], ap["t_o"], ap["t_ps"],
                                    ap["t_acc"], ap["t_den"])
    kmax = 4 * qb + 4
    ots = [t_acc.tile([128, 512], F32, name=f"ot_{qb}_{j}", tag=f"ot{j}")
           for j in range(HL)]
    dens = t_den.tile([128, 512], F32, name=f"den_{qb}", tag="den")
    nc.vector.memset(dens[:], 0.0)
    for kk in range(kmax):
        o = kk - 4 * qb
        c0 = max(0, o) * 128
        pts = []
        for j in range(HL):
            sT = t_ps.tile([128, 512], F32, name=f"sT_{qb}_{j}_{kk}", tag="sT")
            nc.tensor.matmul(sT[:, c0:512],
                             ktv[:, j, kk * 128:(kk + 1) * 128],
                             qtv[:, j, qb * 512 + c0:(qb + 1) * 512],
                             start=True, stop=False)
            b = 64 * (j % 2)
            nc.tensor.matmul(sT[:, c0:512],
                             kpe2[b:b + 64, kk * 128:(kk + 1) * 128],
                             qtv[b:b + 64, 4 + j // 2,
                                 qb * 512 + c0:(qb + 1) * 512],
                             start=False, stop=True)
            pT = t_p.tile([128, 512], BF16, name=f"pT_{qb}_{j}_{kk}",
                          tag=f"pT{j}")
            nc.scalar.activation(pT[:, c0:512], sT[:, c0:512], AF.Exp)
            if o >= 0:
                nc.vector.tensor_tensor(pT[:, c0:c0 + 128],
                                        pT[:, c0:c0 + 128], tri_b[:], op=MUL)
            pts.append(pT)
        for j in range(HL):
            nc.tensor.matmul(dens[32 * j:32 * j + 1, c0:512], ones_b,
                             pts[j][:, c0:512], start=(kk == 0),
                             stop=(kk == kmax - 1), tile_position=(0, 32 * j))
        for j in range(HL):
            nc.tensor.matmul(ots[j][:, c0:512],
                             vv[:, kk, j * 128:(j + 1) * 128],
                             pts[j][:, c0:512], start=(kk == 0),
                             stop=(kk == kmax - 1))
    rd = t_o.tile([97, 512], F32R, name=f"rd_{qb}", tag="rd")
    with nc.allow_low_precision(reason="f32r = fp32 bits"):
        nc.vector.reciprocal(rd[:], dens[0:97, :])
    for j in range(HL):
        bcp = t_ps.tile([128, 512], F32, name=f"bcp_{qb}_{j}", tag="sT")
        nc.tensor.matmul(bcp[:], ones_r[32 * j:32 * j + 1, :],
                         rd[32 * j:32 * j + 1, :], start=True, stop=True,
                         tile_position=(32 * j, 0))
        bcs = t_o.tile([128, 512], F32R, name=f"bcs_{qb}_{j}", tag=f"bcs{j % 2}")
        nc.vector.tensor_copy(bcs[:], bcp[:])
        obf = t_o.tile([128, 512], BF16, name=f"obf_{qb}_{j}", tag=f"obf{j % 2}")
        nc.vector.tensor_tensor(obf[:], ots[j][:], bcs[:], op=MUL)
        nc.sync.dma_start(ag2_in[j * 128:(j + 1) * 128, :], obf[:])
    nc.gpsimd.collective_compute(
        "AllGather", mybir.AluOpType.bypass,
        ins=[ag2_in[:]], outs=[ag2_out[:]],
        replica_groups=[list(range(NCORES))],
    )


def _po_block(nc, tc, tq, io, ag2_out, wov, ap):
    """Output projection for one 512-token block (D-column split)."""
    po_a, po_st, po_ps = ap["po_a"], ap["po_st"], ap["po_ps"]
    oa = po_a.tile([128, 32 * 512], BF16, name=f"oa_{tq}", tag="oa")
    oav = oa[:].rearrange("p (k t) -> p k t", k=32)
    nc.sync.dma_start(oav, ag2_out[:].rearrange("(k p) t -> p k t", p=128))
    for d in range(5):
        ps = po_ps.tile([128, 512], F32, name=f"ops_{tq}_{d}", tag="ops")
        for k in range(32):
            nc.tensor.matmul(ps[:], wov[:, k, d * 128:(d + 1) * 128],
                             oav[:, k, :], start=(k == 0), stop=(k == 31))
        st = po_st.tile([128, 512], F32, name=f"ost_{tq}_{d}", tag="ost")
        nc.vector.tensor_copy(st[:], ps[:])
        nc.sync.dma_start(
            io["outT"][d * 128:(d + 1) * 128, tq * 512:(tq + 1) * 512], st[:])


def _build():
    nc = bass.Bass("TRN2", target_bir_lowering=False, debug=False,
                   num_devices=NCORES)
    io = {
        "hT": nc.dram_tensor("hT", [D, TC], BF16, kind="ExternalInput"),
        "wqa": nc.dram_tensor("wqa", [D, QR], BF16, kind="ExternalInput"),
        "wkva": nc.dram_tensor("wkva", [D, LAT], BF16, kind="ExternalInput"),
        "biask": nc.dram_tensor("biask", [128, 5], F32, kind="ExternalInput"),
        "wqb": nc.dram_tensor("wqb", [QR, 6 * 128], BF16, kind="ExternalInput"),
        "wkvbk": nc.dram_tensor("wkvbk", [KVR, HL * NOPE], BF16,
                                kind="ExternalInput"),
        "wkvbv": nc.dram_tensor("wkvbv", [KVR, HL * VH], BF16,
                                kind="ExternalInput"),
        "wo": nc.dram_tensor("wo", [H * VH, DCOL], BF16, kind="ExternalInput"),
        "cosT": nc.dram_tensor("cosT", [128, T], F32R, kind="ExternalInput"),
        "sinT": nc.dram_tensor("sinT", [128, T], F32R, kind="ExternalInput"),
        "cosA": nc.dram_tensor("cosA", [128, TC], F32R, kind="ExternalInput"),
        "sinA": nc.dram_tensor("sinA", [128, TC], F32R, kind="ExternalInput"),
        "tri": nc.dram_tensor("tri", [128, 128], F32R, kind="ExternalInput"),
        "onesin": nc.dram_tensor("onesin", [128, 128], F32R, kind="ExternalInput"),
        "outT": nc.dram_tensor("outT", [DCOL, T], F32, kind="ExternalOutput"),
    }

    with TileContext(nc) as tc:
        with (
            tc.tile_pool(name="dram", bufs=1, space="DRAM") as dram,
            tc.tile_pool(name="consts", bufs=1) as consts,
            tc.tile_pool(name="persist", bufs=1) as persist,
        ):
            ag1kv_in = dram.tile([LAT, TC], BF16, name="ag1kv_in")
            ag1kv_out = dram.tile([NCORES * LAT, TC], BF16, addr_space="Shared",
                                  name="ag1kv_out")
            ag1q_in = dram.tile([QR, TC], BF16, name="ag1q_in")
            ag1q_out = dram.tile([NCORES * QR, TC], BF16, addr_space="Shared",
                                 name="ag1q_out")
            ag2_ins = [dram.tile([HL * VH, 512], BF16, name=f"ag2_in_{qb}")
                       for qb in range(4)]
            ag2_outs = [dram.tile([H * VH, 512], BF16, addr_space="Shared",
                                  name=f"ag2_out_{qb}") for qb in range(4)]

            consts_t = {}
            ones_sb = consts.tile([128, 128], F32R, name="ones_sb")
            nc.sync.dma_start(ones_sb[:], io["onesin"][:])
            consts_t["ones_c"] = ones_sb[:, 0:1]
            consts_t["ones_r"] = ones_sb
            ones_b = consts.tile([128, 1], BF16, name="ones_b")
            nc.vector.tensor_copy(ones_b[:], ones_sb[:, 0:1])
            consts_t["ones_cb"] = ones_b[:]
            trib = consts.tile([128, 128], BF16, name="trib")
            consts_t["tri_b"] = trib
            for nm, srcn, shp in (("tri_sb", "tri", [128, 128]),
                                  ("cosa_sb", "cosA", [128, TC]),
                                  ("sina_sb", "sinA", [128, TC]),
                                  ):
                consts_t[nm] = consts.tile(shp, F32R, name=nm)
                nc.sync.dma_start(consts_t[nm][:], io[srcn][:])
            consts_t["bias_sb"] = consts.tile([128, 5], F32, name="bias_sb")
            nc.sync.dma_start(consts_t["bias_sb"][:], io["biask"][:])
            nc.vector.tensor_copy(trib[:], consts_t["tri_sb"][:])

            # long-lived attention operands
            kt_sb = persist.tile([128, HL * T], BF16, name="kt_sb")
            ktv = kt_sb[:].rearrange("p (j t) -> p j t", j=HL)
            v_sb = persist.tile([128, 16 * 512], BF16, name="v_sb")
            vv = v_sb[:].rearrange("p (mt c) -> p mt c", mt=16)
            kpe2 = persist.tile([128, T], BF16, name="kpe2")
            qt_sb = persist.tile([128, 6 * T], BF16, name="qt_sb")
            qtv = qt_sb[:].rearrange("p (m t) -> p m t", m=6)
            wo_sb = persist.tile([128, 32 * DCOL], BF16, name="wo_sb")
            wov = wo_sb[:].rearrange("p (k c) -> p k c", k=32)
            nc.sync.dma_start(
                wov, io["wo"][:].rearrange("(k p) c -> p k c", p=128))

            with (
                tc.tile_pool(name="a_ht", bufs=1) as a_ht,
                tc.tile_pool(name="a_w", bufs=3) as a_w,
                tc.tile_pool(name="a_st", bufs=1) as a_st,
                tc.tile_pool(name="a_tmp", bufs=3) as a_tmp,
                tc.tile_pool(name="a_ps", bufs=2, space="PSUM") as a_ps,
                tc.tile_pool(name="a_ss", bufs=1, space="PSUM") as a_ss,
            ):
                ht_sb = a_ht.tile([128, 40 * TC], BF16, name="ht_sb")
                htv = ht_sb[:].rearrange("p (k t) -> p k t", k=40)
                nc.sync.dma_start(
                    htv, io["hT"][:].rearrange("(k p) t -> p k t", p=128))
                stage = a_st.tile([128, 17 * TC], F32R, name="stage")
                ah = {"htv": htv, "stage": stage, "a_w": a_w, "a_ps": a_ps,
                      "a_tmp": a_tmp,
                      "ss_q": a_ss.tile([1, TC], F32, name="ss_q"),
                      "ss_kv": a_ss.tile([1, TC], F32, name="ss_kv")}
                with nc.named_scope("phase_a_kv"):
                    _phase_a_kv(nc, tc, io, consts_t, ah, ag1kv_in)
                with nc.named_scope("ag1kv"):
                    nc.gpsimd.collective_compute(
                        "AllGather", mybir.AluOpType.bypass,
                        ins=[ag1kv_in[:]], outs=[ag1kv_out[:]],
                        replica_groups=[list(range(NCORES))],
                    )
                with nc.named_scope("phase_a_q"):
                    _phase_a_q(nc, tc, io, consts_t, ah, ag1q_in)
                with nc.named_scope("ag1q"):
                    nc.gpsimd.collective_compute(
                        "AllGather", mybir.AluOpType.bypass,
                        ins=[ag1q_in[:]], outs=[ag1q_out[:]],
                        replica_groups=[list(range(NCORES))],
                    )

            ag1kvv = ag1kv_out[:].rearrange("(r a) t -> a r t", a=LAT)
            ag1qv = ag1q_out[:].rearrange("(r a) t -> a r t", a=QR)

            with nc.named_scope("phase_b"):
                _phase_b(nc, tc, io, ag1kvv, ktv, vv, kpe2)
            with nc.named_scope("phase_q"):
                _phase_q(nc, tc, io, ag1qv, qtv)

            with (
                tc.tile_pool(name="t_p", bufs=2) as t_p,
                tc.tile_pool(name="t_o", bufs=2) as t_o,
                tc.tile_pool(name="t_ps", bufs=2, space="PSUM") as t_ps,
                tc.tile_pool(name="t_acc", bufs=1, space="PSUM") as t_acc,
                tc.tile_pool(name="t_den", bufs=1, space="PSUM") as t_den,
                tc.tile_pool(name="po_a", bufs=2) as po_a,
                tc.tile_pool(name="po_st", bufs=3) as po_st,
                tc.tile_pool(name="po_ps", bufs=1, space="PSUM") as po_ps,
            ):
                ap = {"t_p": t_p, "t_o": t_o, "t_ps": t_ps, "t_acc": t_acc,
                      "t_den": t_den, "po_a": po_a, "po_st": po_st,
                      "po_ps": po_ps}
                with nc.named_scope("phase_attn"):
                    for qb in range(4):
                        _attn_block(nc, tc, qb, qtv, ktv, vv, kpe2,
                                    ag2_ins[qb], ag2_outs[qb], consts_t, ap)
                        if qb >= 1:
                            with nc.named_scope("phase_out"):
                                _po_block(nc, tc, qb - 1, io, ag2_outs[qb - 1],
                                          wov, ap)
                    with nc.named_scope("phase_out"):
                        _po_block(nc, tc, 3, io, ag2_outs[3], wov, ap)
    return nc


def _get_nc():
    if "nc" not in _cache:
        _cache["nc"] = _build()
    return _cache["nc"]


def _prep(inputs):
    h = np.asarray(inputs["h"], np.float32)
    pos = np.asarray(inputs["position_ids"], np.int32)
    Wq_a = np.asarray(inputs["Wq_a"], np.float32)
    gq = np.asarray(inputs["gq"], np.float32)
    Wq_b = np.asarray(inputs["Wq_b"], np.float32)
    Wkv_a = np.asarray(inputs["Wkv_a"], np.float32)
    bkv_a = np.asarray(inputs["bkv_a"], np.float32)
    gkv = np.asarray(inputs["gkv"], np.float32)
    Wkv_b = np.asarray(inputs["Wkv_b"], np.float32)
    Wo = np.asarray(inputs["Wo"], np.float32)

    dperm = np.concatenate([np.arange(0, ROPE, 2), np.arange(1, ROPE, 2)])
    scale = np.float32(1.0 / math.sqrt(QK))

    hT = np.ascontiguousarray(h.T)                      # [D, T]
    wkva = Wkv_a.copy()
    wkva[:, KVR:] = Wkv_a[:, KVR + dperm]
    bias = bkv_a.copy()
    bias[KVR:] = bkv_a[KVR + dperm]
    bm = np.zeros((5, 128), np.float32)
    bm.reshape(-1)[:LAT] = bias
    biask = np.ascontiguousarray(bm.T)                  # [128, 5]

    wqb_eff = (Wq_b * gq[:, None]) * scale              # [QR, H*QK]
    wkvb_eff = Wkv_b * gkv[:, None]                     # [KVR, H*(NOPE+VH)]

    inv = THETA ** (-np.arange(0, ROPE, 2, dtype=np.float32) / ROPE)
    fr = pos.astype(np.float32)[:, None] * inv[None, :]  # [T, 32]
    cosT = np.ascontiguousarray(np.tile(np.cos(fr).T, (4, 1)))  # [128, T]
    sinT = np.ascontiguousarray(np.tile(np.sin(fr).T, (4, 1)))
    tri = np.triu(np.ones((128, 128), np.float32))
    wqa_b = Wq_a.astype(ml_dtypes.bfloat16)
    wkva_b = wkva.astype(ml_dtypes.bfloat16)

    bf16 = ml_dtypes.bfloat16
    in_maps = []
    for c in range(NCORES):
        heads = list(range(HL * c, HL * (c + 1)))
        qcols = [np.arange(hh * QK, hh * QK + NOPE) for hh in heads]
        for pair in range(2):
            for hh in heads[2 * pair:2 * pair + 2]:
                qcols.append(hh * QK + NOPE + dperm)
        kcols = np.concatenate(
            [np.arange(hh * (NOPE + VH), hh * (NOPE + VH) + NOPE)
             for hh in heads])
        vcols = np.concatenate(
            [np.arange(hh * (NOPE + VH) + NOPE, (hh + 1) * (NOPE + VH))
             for hh in heads])
        in_maps.append({
            "hT": np.ascontiguousarray(hT[:, c * TC:(c + 1) * TC]).astype(bf16),
            "wqa": wqa_b,
            "wkva": wkva_b,
            "biask": biask,
            "wqb": np.ascontiguousarray(wqb_eff[:, np.concatenate(qcols)]).astype(bf16),
            "wkvbk": np.ascontiguousarray(wkvb_eff[:, kcols]).astype(bf16),
            "wkvbv": np.ascontiguousarray(wkvb_eff[:, vcols]).astype(bf16),
            "wo": np.ascontiguousarray(Wo[:, c * DCOL:(c + 1) * DCOL]).astype(bf16),
            "cosT": cosT,
            "sinT": sinT,
            "cosA": np.ascontiguousarray(cosT[:, c * TC:(c + 1) * TC]),
            "sinA": np.ascontiguousarray(sinT[:, c * TC:(c + 1) * TC]),
            "tri": tri,
            "onesin": np.ones((128, 128), np.float32),
        })
    return in_maps


def kernel(**inputs):
    nc = _get_nc()
    in_maps = _prep(inputs)
    res = bass_utils.run_bass_kernel_spmd(
        nc, in_maps, core_ids=list(range(NCORES)), trace=TRACE[0])
    LAST_RESULT[0] = res
    out = np.empty((T, D), np.float32)
    for c in range(NCORES):
        out[:, c * DCOL:(c + 1) * DCOL] = res.results[c]["outT"].T
    return out
